# revision 1
# baseline (speedup 1.0000x reference)
"""Trainium2 Bass kernel for BaseAttention (Bahdanau-style additive attention).

Reference computation (per batch row b):
    att_h  = h @ W.T + b_h                         # [B, A]
    dot    = tanh(iaf + att_h[:, None, :])         # [B, L, A]
    scores = dot @ alpha + alpha_b                 # [B, L]
    w      = softmax(scores, axis=1)               # [B, L]
    out    = sum_l w[b, l] * af[b, l, :]           # [B, D]

Sharding: data-parallel over batch, B=128 -> 16 per core across 8 cores.

Per-core device layout (natural row-major, rows = (b, l) flattened, R=3136):
  - iaf [R, A] streamed in [128, A] tiles; att_h broadcast to tile rows via an
    indicator matmul (ind_t.T @ att_hb); add + tanh; scores via DVE
    tensor_tensor_reduce against a pre-broadcast alpha row.
  - softmax denominator deferred: e = exp(scores) unnormalized; the final
    result is (sum_l e*af) * 1/(sum_l e).
  - weighted sum over l is a single matmul per (tile, d-chunk) using masked
    lhsT columns: e_cols[:, b] = e * indicator(row belongs to b); masking makes
    the batched per-b matvec one M=16 matmul. float32r (single-pass fp32)
    keeps the tensor engine at 1x rate.
"""

import os
from contextlib import ExitStack

import numpy as np

import concourse.bass as bass
import concourse.mybir as mybir
import concourse.tile as tile
from concourse import bacc
from concourse.bass_utils import run_bass_kernel_spmd

F32 = mybir.dt.float32
F32R = mybir.dt.float32r
AF_T = mybir.ActivationFunctionType

B, L, D, A = 128, 196, 2048, 512
NCORES = 8
BPC = B // NCORES          # 16 batch rows per core
R = BPC * L                # 3136 (b, l) rows per core
P = 128                    # partitions
NT = (R + P - 1) // P      # 25 row tiles (24 full + one 64-row tail)
GROUP = 5                  # row tiles per DMA super-tile
KCH = D // P               # 16 k-chunks for the h @ W.T matmul
DCH = 4                    # d chunks of 512 for the weighted sum
DC = D // DCH              # 512


def _row_groups():
    """(tile0, ntiles, rows_in_last_tile) per DMA super-tile."""
    groups = []
    t = 0
    while t < NT:
        n = min(GROUP, NT - t)
        rows_last = R - (t + n - 1) * P if (t + n) == NT else P
        groups.append((t, n, rows_last))
        t += n
    return groups


def _build_program():
    nc = bacc.Bacc(None, target_bir_lowering=False)

    h_t = nc.declare_dram_parameter("h_t", [D, BPC], F32R, isOutput=False)
    w_t = nc.declare_dram_parameter("w_t", [D, A], F32R, isOutput=False)
    b_bc = nc.declare_dram_parameter("b_bc", [BPC, A], F32, isOutput=False)
    alpha_bc = nc.declare_dram_parameter("alpha_bc", [P, A], F32, isOutput=False)
    alphab_bc = nc.declare_dram_parameter("alphab_bc", [P, 1], F32, isOutput=False)
    ind = nc.declare_dram_parameter("ind", [NT * P, BPC], F32R, isOutput=False)
    ind_t = nc.declare_dram_parameter("ind_t", [BPC, R], F32R, isOutput=False)
    iaf = nc.declare_dram_parameter("iaf", [R, A], F32, isOutput=False)
    af = nc.declare_dram_parameter("af", [R, D], F32R, isOutput=False)
    out = nc.declare_dram_parameter("out", [BPC, D], F32, isOutput=True)

    with ExitStack() as ctx:
        tc = ctx.enter_context(tile.TileContext(nc))
        consts = ctx.enter_context(tc.tile_pool(name="consts", bufs=1))
        wpool = ctx.enter_context(tc.tile_pool(name="wpool", bufs=1))
        iafp = ctx.enter_context(tc.tile_pool(name="iafp", bufs=1))
        afp = ctx.enter_context(tc.tile_pool(name="afp", bufs=2))
        scr = ctx.enter_context(tc.tile_pool(name="scr", bufs=2))
        ps_bc = ctx.enter_context(
            tc.tile_pool(name="ps_bc", bufs=2, space=bass.MemorySpace.PSUM)
        )
        ps_hb = ctx.enter_context(
            tc.tile_pool(name="ps_hb", bufs=1, space=bass.MemorySpace.PSUM)
        )
        ps_acc = ctx.enter_context(
            tc.tile_pool(name="ps_acc", bufs=1, space=bass.MemorySpace.PSUM)
        )

        # --- constants / weights ---
        w_sb = wpool.tile([P, KCH, A], F32R)
        nc.sync.dma_start(w_sb[:], w_t[:, :].rearrange("(k p) a -> p k a", p=P))
        ht_sb = consts.tile([P, KCH, BPC], F32R)
        nc.sync.dma_start(ht_sb[:], h_t[:, :].rearrange("(k p) b -> p k b", p=P))
        bbc_sb = consts.tile([BPC, A], F32)
        nc.sync.dma_start(bbc_sb[:], b_bc[:, :])
        abc_sb = consts.tile([P, A], F32)
        nc.sync.dma_start(abc_sb[:], alpha_bc[:, :])
        abb_sb = consts.tile([P, 1], F32)
        nc.sync.dma_start(abb_sb[:], alphab_bc[:, :])
        ind_sb = consts.tile([P, NT, BPC], F32R)
        nc.sync.dma_start(ind_sb[:], ind[:, :].rearrange("(t p) b -> p t b", p=P))
        indt_sb = consts.tile([BPC, R], F32R)
        nc.sync.dma_start(indt_sb[:], ind_t[:, :])

        scores_all = consts.tile([P, NT], F32)
        e_all = consts.tile([P, NT], F32R)

        # --- att_hb = h @ W.T + b_h, shape [BPC, A] ---
        atthb_ps = ps_hb.tile([BPC, A], F32)
        for k in range(KCH):
            nc.tensor.matmul(
                atthb_ps[:],
                ht_sb[:, k, :],
                w_sb[:, k, :],
                start=(k == 0),
                stop=(k == KCH - 1),
            )
        atthb_sb = consts.tile([BPC, A], F32R)
        nc.vector.tensor_add(atthb_sb[:], atthb_ps[:], bbc_sb[:])

        # --- accumulators for the weighted sum and softmax denominator ---
        acc_ps = ps_acc.tile([BPC, DCH, DC], F32)
        sums_ps = ps_acc.tile([BPC, 1], F32)

        # --- iaf: fully SBUF-resident (6.4 MB), loaded in 4-tile chunks so
        # phase 1 starts as each chunk lands and fully decouples from the
        # af stream ---
        iaf_all = iafp.tile([P, NT, A], F32)
        NFULL_T = R // P  # 24 full tiles
        TAILR = R - NFULL_T * P
        for c in range(0, NFULL_T, 4):
            nc.sync.dma_start(
                iaf_all[:, c : c + 4, :],
                iaf[c * P : (c + 4) * P, :].rearrange("(t p) a -> p t a", p=P),
            )
        nc.sync.dma_start(iaf_all[:TAILR, NFULL_T, :], iaf[NFULL_T * P :, :])

        # --- af stream: 4-tile (4 MB) DMAs on the sync ring ---
        AFG = 4
        af_tiles = {}
        for t in range(NT):
            pt = P if t < NT - 1 else R - (NT - 1) * P
            rt = t * P

            if t % AFG == 0:
                n = min(AFG, NT - t)
                nfull = n
                if t + n == NT and R - (t + n - 1) * P < P:
                    nfull = n - 1
                g = afp.tile([P, AFG, D], F32R, tag="af")
                if nfull:
                    nc.sync.dma_start(
                        g[:, :nfull, :],
                        af[rt : rt + nfull * P, :].rearrange("(t p) d -> p t d", p=P),
                    )
                if nfull < n:
                    rl = R - (NT - 1) * P
                    nc.sync.dma_start(
                        g[:rl, nfull, :], af[rt + nfull * P : rt + nfull * P + rl, :]
                    )
                for jj in range(n):
                    af_tiles[t + jj] = (g, jj)

            af_g, af_j = af_tiles.pop(t)
            iaf_g, iaf_j = iaf_all, t

            # att_hb broadcast to this tile's rows: ind_t[:, rows].T @ att_hb
            bc_ps = ps_bc.tile([P, A], F32, tag="bc")
            nc.tensor.matmul(
                bc_ps[:pt, :],
                indt_sb[:, rt : rt + pt],
                atthb_sb[:],
                start=True,
                stop=True,
            )

            tadd = scr.tile([P, A], F32, tag="tadd")
            nc.vector.tensor_add(tadd[:pt, :], iaf_g[:pt, iaf_j, :], bc_ps[:pt, :])
            tanh = scr.tile([P, A], F32, tag="tanh")
            nc.scalar.activation(tanh[:pt, :], tadd[:pt, :], AF_T.Tanh)

            # scores[:, t] = sum_a tanh * alpha  (alpha_b folded into Exp bias;
            # tensor_tensor_reduce wedges the DVE at runtime here, so use
            # separate mul + reduce)
            ttr_out = scr.tile([P, A], F32, tag="ttr")
            nc.vector.tensor_mul(ttr_out[:pt, :], tanh[:pt, :], abc_sb[:pt, :])
            nc.vector.tensor_reduce(
                scores_all[:pt, t : t + 1],
                ttr_out[:pt, :],
                axis=mybir.AxisListType.X,
                op=mybir.AluOpType.add,
            )
            nc.scalar.activation(
                e_all[:pt, t : t + 1],
                scores_all[:pt, t : t + 1],
                AF_T.Exp,
                bias=abb_sb[:pt, :],
            )

            # masked weight columns: e_cols[:, b] = e * (row belongs to b)
            ecols = scr.tile([P, BPC], F32R, tag="ecols")
            nc.vector.tensor_scalar_mul(
                ecols[:pt, :],
                ind_sb[:pt, t, :].bitcast(F32),
                e_all[:pt, t : t + 1].bitcast(F32),
            )

            for c in range(DCH):
                nc.tensor.matmul(
                    acc_ps[:, c, :],
                    ecols[:pt, :],
                    af_g[:pt, af_j, c * DC : (c + 1) * DC],
                    start=(t == 0),
                    stop=(t == NT - 1),
                )
            # N=1 violates the fp32r even-free-dim ISA rule; plain fp32
            # is fine for this tiny matmul.
            nc.tensor.matmul(
                sums_ps[:],
                ind_sb[:pt, t, :].bitcast(F32),
                e_all[:pt, t : t + 1].bitcast(F32),
                start=(t == 0),
                stop=(t == NT - 1),
            )

        # --- normalize and store ---
        recip = consts.tile([BPC, 1], F32)
        nc.vector.reciprocal(recip[:], sums_ps[:])
        out_sb = consts.tile([BPC, D], F32)
        nc.scalar.mul(
            out_sb[:, :].rearrange("b (c d) -> b c d", c=DCH), acc_ps[:, :, :], recip[:]
        )
        nc.sync.dma_start(out[:, :], out_sb[:])

    nc.compile()
    return nc


_PROGRAM = None


def _get_program():
    global _PROGRAM
    if _PROGRAM is None:
        _PROGRAM = _build_program()
    return _PROGRAM


def _host_prep(h, att_feats, internal_att_feats, h2att_w, h2att_b, alpha_w, alpha_b):
    h = np.asarray(h, np.float32)
    att_feats = np.ascontiguousarray(np.asarray(att_feats, np.float32))
    iaf = np.ascontiguousarray(np.asarray(internal_att_feats, np.float32))
    h2att_w = np.asarray(h2att_w, np.float32)
    h2att_b = np.asarray(h2att_b, np.float32)
    alpha_w = np.asarray(alpha_w, np.float32)
    alpha_b = np.asarray(alpha_b, np.float32)

    w_t = np.ascontiguousarray(h2att_w.T)                      # [D, A]
    b_bc = np.tile(h2att_b.reshape(1, A), (BPC, 1))            # [BPC, A]
    alpha_bc = np.tile(alpha_w.reshape(1, A), (P, 1))          # [P, A]
    alphab_bc = np.full((P, 1), float(alpha_b.reshape(-1)[0]), np.float32)

    ind = np.zeros((NT * P, BPC), np.float32)
    rows = np.arange(R)
    ind[rows, rows // L] = 1.0
    ind_t = np.ascontiguousarray(ind[:R].T)                    # [BPC, R]

    in_maps = []
    for i in range(NCORES):
        sl = slice(i * BPC, (i + 1) * BPC)
        in_maps.append(
            {
                "h_t": np.ascontiguousarray(h[sl].T),
                "w_t": w_t,
                "b_bc": b_bc,
                "alpha_bc": alpha_bc,
                "alphab_bc": alphab_bc,
                "ind": ind,
                "ind_t": ind_t,
                "iaf": iaf[sl].reshape(R, A),
                "af": att_feats[sl].reshape(R, D),
            }
        )
    return in_maps


def run(trace=False, **inputs):
    """Run the SPMD kernel; returns (full_output [B, D], BassKernelResults)."""
    nc = _get_program()
    in_maps = _host_prep(**inputs)
    res = run_bass_kernel_spmd(nc, in_maps, list(range(NCORES)), trace=trace)
    out = np.concatenate([res.results[i]["out"] for i in range(NCORES)], axis=0)
    return out, res


def kernel(**inputs):
    out, _ = run(trace=False, **inputs)
    return out



# revision 3
# speedup vs baseline: 1.6071x; 1.6071x over previous
"""Trainium2 Bass kernel for BaseAttention (Bahdanau-style additive attention).

Reference computation (per batch row b):
    att_h  = h @ W.T + b_h                         # [B, A]
    dot    = tanh(iaf + att_h[:, None, :])         # [B, L, A]
    scores = dot @ alpha + alpha_b                 # [B, L]
    w      = softmax(scores, axis=1)               # [B, L]
    out    = sum_l w[b, l] * af[b, l, :]           # [B, D]

Sharding: data-parallel over batch, B=128 -> 16 per core across 8 cores.

The kernel is HBM-bound, so every large tensor is downcast to bf16 on the
host (rel-err budget is 2e-2; bf16 costs ~2e-3) and pre-packed into
partition-major layouts so each DMA line is one long contiguous run:
  - af  [P, NT*D]  : af_dev[p, t*D+d] = af[t*P+p, d]   (20 KB lines/group)
  - iaf [P, NT*A]  : same row tiling                   (5 KB lines/chunk)
  - w   [P, KCH*A] : w_dev[p, k*A+a] = W[a, k*P+p]
  - h_t [P, KCH*BPC], ind [P, NT*BPC] similarly packed (no 64 B packets)

Per 128-row tile t (rows = (b, l) flattened, padded to NT*P):
  - att_h broadcast to tile rows via an indicator matmul (ind_t.T @ att_hb)
    into PSUM; add iaf + tanh + alpha-mul + reduce gives scores; softmax
    denominator deferred: e = exp(scores) unnormalized, final result is
    (sum_l e*af) * 1/(sum_l e).
  - weighted sum over l is a matmul per (tile, d-chunk) using masked lhsT
    columns: e_cols[:, b] = e * indicator(row belongs to b).
  - denominator via the same e_cols against a constant [1, 0] column pair
    (free dim 2 keeps the 16-bit matmul free-dim rule happy).
"""

import os
from contextlib import ExitStack

import numpy as np
import ml_dtypes

import concourse.bass as bass
import concourse.mybir as mybir
import concourse.tile as tile
from concourse import bacc
from concourse.bass_utils import run_bass_kernel_spmd

F32 = mybir.dt.float32
BF16 = mybir.dt.bfloat16
AF_T = mybir.ActivationFunctionType
NPBF16 = ml_dtypes.bfloat16

B, L, D, A = 128, 196, 2048, 512
NCORES = 8
BPC = B // NCORES          # 16 batch rows per core
R = BPC * L                # 3136 (b, l) rows per core
P = 128                    # partitions
NT = (R + P - 1) // P      # 25 row tiles (24 full + one 64-row tail)
NFULL_T = R // P           # 24
TAILR = R - NFULL_T * P    # 64
KCH = D // P               # 16 k-chunks for the h @ W.T matmul
DCH = 4                    # d chunks of 512 for the weighted sum
DC = D // DCH              # 512
AFG = 5                    # row tiles per af/iaf DMA group (25 = 5*5)


def _build_program():
    nc = bacc.Bacc(None, target_bir_lowering=False)

    h_t = nc.declare_dram_parameter("h_t", [P, KCH * BPC], BF16, isOutput=False)
    w_t = nc.declare_dram_parameter("w_t", [P, KCH * A], BF16, isOutput=False)
    b_bc = nc.declare_dram_parameter("b_bc", [BPC, A], BF16, isOutput=False)
    alpha_bc = nc.declare_dram_parameter("alpha_bc", [P, A], BF16, isOutput=False)
    alphab_bc = nc.declare_dram_parameter("alphab_bc", [P, 1], F32, isOutput=False)
    ind = nc.declare_dram_parameter("ind", [P, NT * BPC], BF16, isOutput=False)
    ind_t = nc.declare_dram_parameter("ind_t", [BPC, R], BF16, isOutput=False)
    iaf = nc.declare_dram_parameter("iaf", [P, NT * A], BF16, isOutput=False)
    af = nc.declare_dram_parameter("af", [P, NT * D], BF16, isOutput=False)
    out = nc.declare_dram_parameter("out", [BPC, D], F32, isOutput=True)

    with ExitStack() as ctx:
        tc = ctx.enter_context(tile.TileContext(nc))
        consts = ctx.enter_context(tc.tile_pool(name="consts", bufs=1))
        wpool = ctx.enter_context(tc.tile_pool(name="wpool", bufs=1))
        iafp = ctx.enter_context(tc.tile_pool(name="iafp", bufs=1))
        afp = ctx.enter_context(tc.tile_pool(name="afp", bufs=2))
        scr = ctx.enter_context(tc.tile_pool(name="scr", bufs=2))
        ps_bc = ctx.enter_context(
            tc.tile_pool(name="ps_bc", bufs=2, space=bass.MemorySpace.PSUM)
        )
        ps_hb = ctx.enter_context(
            tc.tile_pool(name="ps_hb", bufs=1, space=bass.MemorySpace.PSUM)
        )
        ps_acc = ctx.enter_context(
            tc.tile_pool(name="ps_acc", bufs=1, space=bass.MemorySpace.PSUM)
        )

        # --- constants / weights (all partition-contiguous in DRAM) ---
        ht_sb = consts.tile([P, KCH * BPC], BF16)
        nc.sync.dma_start(ht_sb[:], h_t[:, :])
        w_sb = wpool.tile([P, KCH * A], BF16)
        nc.sync.dma_start(w_sb[:], w_t[:, :])
        bbc_sb = consts.tile([BPC, A], BF16)
        nc.sync.dma_start(bbc_sb[:], b_bc[:, :])
        abc_sb = consts.tile([P, A], BF16)
        nc.sync.dma_start(abc_sb[:], alpha_bc[:, :])
        abb_sb = consts.tile([P, 1], F32)
        nc.sync.dma_start(abb_sb[:], alphab_bc[:, :])
        ind_sb = consts.tile([P, NT * BPC], BF16)
        nc.sync.dma_start(ind_sb[:], ind[:, :])
        indt_sb = consts.tile([BPC, R], BF16)
        nc.sync.dma_start(indt_sb[:], ind_t[:, :])

        ones2_sb = consts.tile([P, 2], BF16)
        nc.gpsimd.memset(ones2_sb[:, 0:1], 1.0)
        nc.gpsimd.memset(ones2_sb[:, 1:2], 0.0)

        scores_all = consts.tile([P, NT], F32)
        e_all = consts.tile([P, NT], F32)

        # --- att_hb = h @ W.T + b_h, shape [BPC, A] ---
        atthb_ps = ps_hb.tile([BPC, A], F32)
        for k in range(KCH):
            nc.tensor.matmul(
                atthb_ps[:],
                ht_sb[:, k * BPC : (k + 1) * BPC],
                w_sb[:, k * A : (k + 1) * A],
                start=(k == 0),
                stop=(k == KCH - 1),
            )
        atthb_sb = consts.tile([BPC, A], BF16)
        nc.vector.tensor_add(atthb_sb[:], atthb_ps[:], bbc_sb[:])

        # --- accumulators for the weighted sum and softmax denominator ---
        acc_ps = ps_acc.tile([BPC, DCH, DC], F32)
        sums_ps = ps_acc.tile([BPC, 2], F32)

        # --- iaf: fully SBUF-resident (3.2 MB bf16), loaded in 5-tile chunks
        # interleaved with the af groups so early tiles are ready fast ---
        iaf_all = iafp.tile([P, NT * A], BF16)

        af_tiles = {}
        for t in range(NT):
            pt = P if t < NT - 1 else TAILR
            rt = t * P

            if t % AFG == 0:
                g = t // AFG
                nc.sync.dma_start(
                    iaf_all[:, t * A : (t + AFG) * A], iaf[:, t * A : (t + AFG) * A]
                )
                af_g = afp.tile([P, AFG * D], BF16, tag="af")
                nc.sync.dma_start(af_g[:], af[:, t * D : (t + AFG) * D])
                for jj in range(AFG):
                    af_tiles[t + jj] = (af_g, jj)

            af_g, af_j = af_tiles.pop(t)

            # att_hb broadcast to this tile's rows: ind_t[:, rows].T @ att_hb
            bc_ps = ps_bc.tile([P, A], F32, tag="bc")
            nc.tensor.matmul(
                bc_ps[:pt, :],
                indt_sb[:, rt : rt + pt],
                atthb_sb[:],
                start=True,
                stop=True,
            )

            tadd = scr.tile([P, A], BF16, tag="tadd")
            nc.vector.tensor_add(
                tadd[:pt, :], iaf_all[:pt, t * A : (t + 1) * A], bc_ps[:pt, :]
            )
            tanh = scr.tile([P, A], BF16, tag="tanh")
            nc.scalar.activation(tanh[:pt, :], tadd[:pt, :], AF_T.Tanh)

            # scores[:, t] = sum_a tanh * alpha  (alpha_b folded into Exp bias;
            # tensor_tensor_reduce wedges the DVE at runtime here, so use
            # separate mul + reduce)
            ttr_out = scr.tile([P, A], BF16, tag="ttr")
            nc.vector.tensor_mul(ttr_out[:pt, :], tanh[:pt, :], abc_sb[:pt, :])
            nc.vector.tensor_reduce(
                scores_all[:pt, t : t + 1],
                ttr_out[:pt, :],
                axis=mybir.AxisListType.X,
                op=mybir.AluOpType.add,
            )
            nc.scalar.activation(
                e_all[:pt, t : t + 1],
                scores_all[:pt, t : t + 1],
                AF_T.Exp,
                bias=abb_sb[:pt, :],
            )

            # masked weight columns: e_cols[:, b] = e * (row belongs to b)
            ecols = scr.tile([P, BPC], BF16, tag="ecols")
            nc.vector.tensor_scalar_mul(
                ecols[:pt, :],
                ind_sb[:pt, t * BPC : (t + 1) * BPC],
                e_all[:pt, t : t + 1],
            )

            for c in range(DCH):
                nc.tensor.matmul(
                    acc_ps[:, c, :],
                    ecols[:pt, :],
                    af_g[:pt, af_j * D + c * DC : af_j * D + (c + 1) * DC],
                    start=(t == 0),
                    stop=(t == NT - 1),
                )
            # denominator: sums[b] = sum_rows e_cols[:, b]; the [1, 0] column
            # pair keeps the 16-bit moving free dim even.
            nc.tensor.matmul(
                sums_ps[:],
                ecols[:pt, :],
                ones2_sb[:pt, :],
                start=(t == 0),
                stop=(t == NT - 1),
            )

        # --- normalize and store ---
        recip = consts.tile([BPC, 1], F32)
        nc.vector.reciprocal(recip[:], sums_ps[:, 0:1])
        out_sb = consts.tile([BPC, D], F32)
        nc.scalar.mul(
            out_sb[:, :].rearrange("b (c d) -> b c d", c=DCH), acc_ps[:, :, :], recip[:]
        )
        nc.sync.dma_start(out[:, :], out_sb[:])

    nc.compile()
    return nc


_PROGRAM = None


def _get_program():
    global _PROGRAM
    if _PROGRAM is None:
        _PROGRAM = _build_program()
    return _PROGRAM


def _pack_rows(x16, ncols):
    """[R, C] bf16 -> [P, NT*C] with dev[p, t*C:(t+1)*C] = x[t*P+p], zero pad."""
    dev = np.zeros((P, NT, ncols), NPBF16)
    dev[:, :NFULL_T, :] = x16[: NFULL_T * P].reshape(NFULL_T, P, ncols).transpose(1, 0, 2)
    dev[:TAILR, NFULL_T, :] = x16[NFULL_T * P :]
    return np.ascontiguousarray(dev.reshape(P, NT * ncols))


def _host_prep(h, att_feats, internal_att_feats, h2att_w, h2att_b, alpha_w, alpha_b):
    h16 = np.asarray(h).astype(NPBF16)
    af16 = np.asarray(att_feats).astype(NPBF16).reshape(B, L * D)
    iaf16 = np.asarray(internal_att_feats).astype(NPBF16).reshape(B, L * A)
    w16 = np.asarray(h2att_w).astype(NPBF16)                    # [A, D]
    h2att_b = np.asarray(h2att_b, np.float32)
    alpha_w = np.asarray(alpha_w, np.float32)
    alpha_b = np.asarray(alpha_b, np.float32)

    # w_dev[p, k*A+a] = W[a, k*P+p]
    w_dev = np.ascontiguousarray(
        w16.T.reshape(KCH, P, A).transpose(1, 0, 2).reshape(P, KCH * A)
    )
    b_bc = np.tile(h2att_b.reshape(1, A).astype(NPBF16), (BPC, 1))
    alpha_bc = np.tile(alpha_w.reshape(1, A).astype(NPBF16), (P, 1))
    alphab_bc = np.full((P, 1), float(alpha_b.reshape(-1)[0]), np.float32)

    ind_rows = np.zeros((NT * P, BPC), np.float32)
    rows = np.arange(R)
    ind_rows[rows, rows // L] = 1.0
    ind_dev = np.ascontiguousarray(
        ind_rows.reshape(NT, P, BPC).transpose(1, 0, 2).reshape(P, NT * BPC)
    ).astype(NPBF16)
    ind_t = np.ascontiguousarray(ind_rows[:R].T).astype(NPBF16)  # [BPC, R]

    in_maps = []
    for i in range(NCORES):
        sl = slice(i * BPC, (i + 1) * BPC)
        # h_t_dev[p, k*BPC+b] = h[b, k*P+p]
        ht_dev = np.ascontiguousarray(
            h16[sl].T.reshape(KCH, P, BPC).transpose(1, 0, 2).reshape(P, KCH * BPC)
        )
        in_maps.append(
            {
                "h_t": ht_dev,
                "w_t": w_dev,
                "b_bc": b_bc,
                "alpha_bc": alpha_bc,
                "alphab_bc": alphab_bc,
                "ind": ind_dev,
                "ind_t": ind_t,
                "iaf": _pack_rows(iaf16[sl].reshape(R, A), A),
                "af": _pack_rows(af16[sl].reshape(R, D), D),
            }
        )
    return in_maps


def run(trace=False, **inputs):
    """Run the SPMD kernel; returns (full_output [B, D], BassKernelResults)."""
    nc = _get_program()
    in_maps = _host_prep(**inputs)
    res = run_bass_kernel_spmd(nc, in_maps, list(range(NCORES)), trace=trace)
    out = np.concatenate([res.results[i]["out"] for i in range(NCORES)], axis=0)
    return out, res


def kernel(**inputs):
    out, _ = run(trace=False, **inputs)
    return out


# revision 4
# speedup vs baseline: 1.7308x; 1.0770x over previous
"""Trainium2 Bass kernel for BaseAttention (Bahdanau-style additive attention).

Reference computation (per batch row b):
    att_h  = h @ W.T + b_h                         # [B, A]
    dot    = tanh(iaf + att_h[:, None, :])         # [B, L, A]
    scores = dot @ alpha + alpha_b                 # [B, L]
    w      = softmax(scores, axis=1)               # [B, L]
    out    = sum_l w[b, l] * af[b, l, :]           # [B, D]

Sharding: data-parallel over batch, B=128 -> 16 per core across 8 cores.

The kernel is HBM-bound, so every large tensor is downcast to bf16 on the
host (rel-err budget is 2e-2; bf16 costs ~3e-3) and pre-packed into
partition-major layouts so each DMA line is one long contiguous run:
  - af  [P, NT*D]  : af_dev[p, t*D+d] = af[t*P+p, d]   (20 KB lines/group)
  - iaf [P, NT*A]  : same row tiling                   (5 KB lines/chunk)
  - w   [P, KCH*A] : w_dev[p, k*A+a] = W[a, k*P+p]
  - h_t [P, KCH*BPC], ind [P, NT*BPC] similarly packed (no 64 B packets)

Per 128-row tile t (rows = (b, l) flattened, padded to NT*P):
  - att_h broadcast to tile rows via an indicator matmul (ind_t.T @ att_hb)
    into PSUM; iaf is added in the same PSUM accumulation group by streaming
    it through the PE behind an identity lhsT (keeps the add off the DVE).
  - tanh straight out of PSUM; alpha-mul + reduce on DVE gives scores;
    softmax denominator deferred: e = exp(scores) unnormalized, the final
    result is (sum_l e*af) * 1/(sum_l e).
  - weighted sum over l is a matmul per (tile, d-chunk) using masked lhsT
    columns: e_cols[:, b] = e * indicator(row belongs to b); the denominator
    reuses e_cols against a constant [1, 0] column pair (free dim 2 keeps
    the 16-bit matmul free-dim rule happy).
  - the loop is software-pipelined with a 1-tile skew: tile t+1's PSUM
    broadcast+inject is emitted before tile t's DVE/ACT chain so no engine's
    in-order queue stalls on a cross-engine dependency.
"""

import os
from contextlib import ExitStack

import numpy as np
import ml_dtypes

import concourse.bass as bass
import concourse.mybir as mybir
import concourse.tile as tile
from concourse import bacc
from concourse.bass_utils import run_bass_kernel_spmd

F32 = mybir.dt.float32
BF16 = mybir.dt.bfloat16
AF_T = mybir.ActivationFunctionType
NPBF16 = ml_dtypes.bfloat16

B, L, D, A = 128, 196, 2048, 512
NCORES = 8
BPC = B // NCORES          # 16 batch rows per core
R = BPC * L                # 3136 (b, l) rows per core
P = 128                    # partitions
NT = (R + P - 1) // P      # 25 row tiles (24 full + one 64-row tail)
NFULL_T = R // P           # 24
TAILR = R - NFULL_T * P    # 64
KCH = D // P               # 16 k-chunks for the h @ W.T matmul
DCH = 4                    # d chunks of 512 for the weighted sum
DC = D // DCH              # 512
AFG = 5                    # row tiles per af/iaf DMA group (25 = 5*5)


def _build_program():
    nc = bacc.Bacc(None, target_bir_lowering=False)

    h_t = nc.declare_dram_parameter("h_t", [P, KCH * BPC], BF16, isOutput=False)
    w_t = nc.declare_dram_parameter("w_t", [P, KCH * A], BF16, isOutput=False)
    b_bc = nc.declare_dram_parameter("b_bc", [BPC, A], BF16, isOutput=False)
    alpha_bc = nc.declare_dram_parameter("alpha_bc", [P, A], BF16, isOutput=False)
    alphab_bc = nc.declare_dram_parameter("alphab_bc", [P, 1], F32, isOutput=False)
    ident = nc.declare_dram_parameter("ident", [P, P], BF16, isOutput=False)
    ind = nc.declare_dram_parameter("ind", [P, NT * BPC], BF16, isOutput=False)
    ind_t = nc.declare_dram_parameter("ind_t", [BPC, R], BF16, isOutput=False)
    iaf = nc.declare_dram_parameter("iaf", [P, NT * A], BF16, isOutput=False)
    af = nc.declare_dram_parameter("af", [P, NT * D], BF16, isOutput=False)
    out = nc.declare_dram_parameter("out", [BPC, D], F32, isOutput=True)

    with ExitStack() as ctx:
        tc = ctx.enter_context(tile.TileContext(nc))
        consts = ctx.enter_context(tc.tile_pool(name="consts", bufs=1))
        wpool = ctx.enter_context(tc.tile_pool(name="wpool", bufs=1))
        iafp = ctx.enter_context(tc.tile_pool(name="iafp", bufs=1))
        afp = ctx.enter_context(tc.tile_pool(name="afp", bufs=3))
        scr = ctx.enter_context(tc.tile_pool(name="scr", bufs=2))
        ps_bc = ctx.enter_context(
            tc.tile_pool(name="ps_bc", bufs=2, space=bass.MemorySpace.PSUM)
        )
        ps_hb = ctx.enter_context(
            tc.tile_pool(name="ps_hb", bufs=1, space=bass.MemorySpace.PSUM)
        )
        ps_acc = ctx.enter_context(
            tc.tile_pool(name="ps_acc", bufs=1, space=bass.MemorySpace.PSUM)
        )

        # --- constants / weights (all partition-contiguous in DRAM) ---
        ht_sb = consts.tile([P, KCH * BPC], BF16)
        nc.sync.dma_start(ht_sb[:], h_t[:, :])
        w_sb = wpool.tile([P, KCH * A], BF16)
        nc.sync.dma_start(w_sb[:], w_t[:, :])
        bbc_sb = consts.tile([BPC, A], BF16)
        nc.sync.dma_start(bbc_sb[:], b_bc[:, :])
        abc_sb = consts.tile([P, A], BF16)
        nc.sync.dma_start(abc_sb[:], alpha_bc[:, :])
        abb_sb = consts.tile([P, 1], F32)
        nc.sync.dma_start(abb_sb[:], alphab_bc[:, :])
        ident_sb = consts.tile([P, P], BF16)
        nc.sync.dma_start(ident_sb[:], ident[:, :])
        ind_sb = consts.tile([P, NT * BPC], BF16)
        nc.sync.dma_start(ind_sb[:], ind[:, :])
        indt_sb = consts.tile([BPC, R], BF16)
        nc.sync.dma_start(indt_sb[:], ind_t[:, :])

        ones2_sb = consts.tile([P, 2], BF16)
        nc.gpsimd.memset(ones2_sb[:, 0:1], 1.0)
        nc.gpsimd.memset(ones2_sb[:, 1:2], 0.0)

        scores_all = consts.tile([P, NT], F32)
        e_all = consts.tile([P, NT], F32)

        # --- iaf: fully SBUF-resident (3.2 MB bf16), loaded in 5-tile chunks
        # interleaved with the af groups so early tiles are ready fast ---
        iaf_all = iafp.tile([P, NT * A], BF16)

        def issue_group(g):
            t0 = g * AFG
            nc.sync.dma_start(
                iaf_all[:, t0 * A : (t0 + AFG) * A], iaf[:, t0 * A : (t0 + AFG) * A]
            )
            af_g = afp.tile([P, AFG * D], BF16, tag="af")
            nc.sync.dma_start(af_g[:], af[:, t0 * D : (t0 + AFG) * D])
            return af_g

        af_groups = {0: issue_group(0)}

        # --- att_hb = h @ W.T + b_h, shape [BPC, A] ---
        atthb_ps = ps_hb.tile([BPC, A], F32)
        for k in range(KCH):
            nc.tensor.matmul(
                atthb_ps[:],
                ht_sb[:, k * BPC : (k + 1) * BPC],
                w_sb[:, k * A : (k + 1) * A],
                start=(k == 0),
                stop=(k == KCH - 1),
            )
        atthb_sb = consts.tile([BPC, A], BF16)
        nc.vector.tensor_add(atthb_sb[:], atthb_ps[:], bbc_sb[:])

        # --- accumulators for the weighted sum and softmax denominator ---
        acc_ps = ps_acc.tile([BPC, DCH, DC], F32)
        sums_ps = ps_acc.tile([BPC, 2], F32)

        def bc_inject(t, pt):
            """tile rows of att_hb + iaf, accumulated in one PSUM group."""
            bc_ps = ps_bc.tile([P, A], F32, tag="bc")
            nc.tensor.matmul(
                bc_ps[:pt, :],
                indt_sb[:, t * P : t * P + pt],
                atthb_sb[:],
                start=True,
                stop=False,
            )
            nc.tensor.matmul(
                bc_ps[:pt, :],
                ident_sb[:pt, :pt],
                iaf_all[:pt, t * A : (t + 1) * A],
                start=False,
                stop=True,
            )
            return bc_ps

        bc_tiles = {0: bc_inject(0, P)}

        for t in range(NT):
            pt = P if t < NT - 1 else TAILR
            g = t // AFG
            af_g = af_groups.pop(g) if t % AFG == 0 else af_g
            bc_ps = bc_tiles.pop(t)

            if t + 1 < NT and (t + 1) % AFG == 0:
                af_groups[g + 1] = issue_group(g + 1)

            tanh = scr.tile([P, A], BF16, tag="tanh")
            nc.scalar.activation(tanh[:pt, :], bc_ps[:pt, :], AF_T.Tanh)

            if t + 1 < NT:
                bc_tiles[t + 1] = bc_inject(t + 1, P if t + 1 < NT - 1 else TAILR)

            # scores[:, t] = sum_a tanh * alpha  (alpha_b folded into Exp bias;
            # tensor_tensor_reduce wedges the DVE at runtime here, so use
            # separate mul + reduce)
            ttr_out = scr.tile([P, A], BF16, tag="ttr")
            nc.vector.tensor_mul(ttr_out[:pt, :], tanh[:pt, :], abc_sb[:pt, :])
            nc.vector.tensor_reduce(
                scores_all[:pt, t : t + 1],
                ttr_out[:pt, :],
                axis=mybir.AxisListType.X,
                op=mybir.AluOpType.add,
            )
            nc.scalar.activation(
                e_all[:pt, t : t + 1],
                scores_all[:pt, t : t + 1],
                AF_T.Exp,
                bias=abb_sb[:pt, :],
            )

            # masked weight columns: e_cols[:, b] = e * (row belongs to b)
            ecols = scr.tile([P, BPC], BF16, tag="ecols")
            nc.vector.tensor_scalar_mul(
                ecols[:pt, :],
                ind_sb[:pt, t * BPC : (t + 1) * BPC],
                e_all[:pt, t : t + 1],
            )

            for c in range(DCH):
                nc.tensor.matmul(
                    acc_ps[:, c, :],
                    ecols[:pt, :],
                    af_g[:pt, (t % AFG) * D + c * DC : (t % AFG) * D + (c + 1) * DC],
                    start=(t == 0),
                    stop=(t == NT - 1),
                )
            # denominator: sums[b] = sum_rows e_cols[:, b]
            nc.tensor.matmul(
                sums_ps[:],
                ecols[:pt, :],
                ones2_sb[:pt, :],
                start=(t == 0),
                stop=(t == NT - 1),
            )

        # --- normalize and store ---
        recip = consts.tile([BPC, 1], F32)
        nc.vector.reciprocal(recip[:], sums_ps[:, 0:1])
        out_sb = consts.tile([BPC, D], F32)
        nc.scalar.mul(
            out_sb[:, :].rearrange("b (c d) -> b c d", c=DCH), acc_ps[:, :, :], recip[:]
        )
        nc.sync.dma_start(out[:, :], out_sb[:])

    nc.compile()
    return nc


_PROGRAM = None


def _get_program():
    global _PROGRAM
    if _PROGRAM is None:
        _PROGRAM = _build_program()
    return _PROGRAM


def _pack_rows(x16, ncols):
    """[R, C] bf16 -> [P, NT*C] with dev[p, t*C:(t+1)*C] = x[t*P+p], zero pad."""
    dev = np.zeros((P, NT, ncols), NPBF16)
    dev[:, :NFULL_T, :] = x16[: NFULL_T * P].reshape(NFULL_T, P, ncols).transpose(1, 0, 2)
    dev[:TAILR, NFULL_T, :] = x16[NFULL_T * P :]
    return np.ascontiguousarray(dev.reshape(P, NT * ncols))


def _host_prep(h, att_feats, internal_att_feats, h2att_w, h2att_b, alpha_w, alpha_b):
    h16 = np.asarray(h).astype(NPBF16)
    af16 = np.asarray(att_feats).astype(NPBF16).reshape(B, L * D)
    iaf16 = np.asarray(internal_att_feats).astype(NPBF16).reshape(B, L * A)
    w16 = np.asarray(h2att_w).astype(NPBF16)                    # [A, D]
    h2att_b = np.asarray(h2att_b, np.float32)
    alpha_w = np.asarray(alpha_w, np.float32)
    alpha_b = np.asarray(alpha_b, np.float32)

    # w_dev[p, k*A+a] = W[a, k*P+p]
    w_dev = np.ascontiguousarray(
        w16.T.reshape(KCH, P, A).transpose(1, 0, 2).reshape(P, KCH * A)
    )
    b_bc = np.tile(h2att_b.reshape(1, A).astype(NPBF16), (BPC, 1))
    alpha_bc = np.tile(alpha_w.reshape(1, A).astype(NPBF16), (P, 1))
    alphab_bc = np.full((P, 1), float(alpha_b.reshape(-1)[0]), np.float32)
    ident = np.eye(P, dtype=NPBF16)

    ind_rows = np.zeros((NT * P, BPC), np.float32)
    rows = np.arange(R)
    ind_rows[rows, rows // L] = 1.0
    ind_dev = np.ascontiguousarray(
        ind_rows.reshape(NT, P, BPC).transpose(1, 0, 2).reshape(P, NT * BPC)
    ).astype(NPBF16)
    ind_t = np.ascontiguousarray(ind_rows[:R].T).astype(NPBF16)  # [BPC, R]

    in_maps = []
    for i in range(NCORES):
        sl = slice(i * BPC, (i + 1) * BPC)
        # h_t_dev[p, k*BPC+b] = h[b, k*P+p]
        ht_dev = np.ascontiguousarray(
            h16[sl].T.reshape(KCH, P, BPC).transpose(1, 0, 2).reshape(P, KCH * BPC)
        )
        in_maps.append(
            {
                "h_t": ht_dev,
                "w_t": w_dev,
                "b_bc": b_bc,
                "alpha_bc": alpha_bc,
                "alphab_bc": alphab_bc,
                "ident": ident,
                "ind": ind_dev,
                "ind_t": ind_t,
                "iaf": _pack_rows(iaf16[sl].reshape(R, A), A),
                "af": _pack_rows(af16[sl].reshape(R, D), D),
            }
        )
    return in_maps


def run(trace=False, **inputs):
    """Run the SPMD kernel; returns (full_output [B, D], BassKernelResults)."""
    nc = _get_program()
    in_maps = _host_prep(**inputs)
    res = run_bass_kernel_spmd(nc, in_maps, list(range(NCORES)), trace=trace)
    out = np.concatenate([res.results[i]["out"] for i in range(NCORES)], axis=0)
    return out, res


def kernel(**inputs):
    out, _ = run(trace=False, **inputs)
    return out


# revision 5
# speedup vs baseline: 1.7735x; 1.0246x over previous
"""Trainium2 Bass kernel for BaseAttention (Bahdanau-style additive attention).

Reference computation (per batch row b):
    att_h  = h @ W.T + b_h                         # [B, A]
    dot    = tanh(iaf + att_h[:, None, :])         # [B, L, A]
    scores = dot @ alpha + alpha_b                 # [B, L]
    w      = softmax(scores, axis=1)               # [B, L]
    out    = sum_l w[b, l] * af[b, l, :]           # [B, D]

Sharding: data-parallel over batch, B=128 -> 16 per core across 8 cores.

The kernel is HBM-bound, so every large tensor is downcast to bf16 on the
host (rel-err budget is 2e-2; bf16 costs ~3e-3) and pre-packed into
partition-major layouts so each DMA line is one long contiguous run:
  - af  [P, NT*D]  : af_dev[p, t*D+d] = af[t*P+p, d]   (20 KB lines/group)
  - iaf [P, NT*A]  : same row tiling                   (5 KB lines/chunk)
  - w   [P, KCH*A] : w_dev[p, k*A+a] = W[a, k*P+p]
  - h_t [P, KCH*BPC], ind [P, NT*BPC] similarly packed (no 64 B packets)

Per 128-row tile t (rows = (b, l) flattened, padded to NT*P):
  - att_h broadcast to tile rows via an indicator matmul (ind_t.T @ att_hb)
    into PSUM; iaf is added in the same PSUM accumulation group by streaming
    it through the PE behind an identity lhsT (keeps the add off the DVE).
  - tanh straight out of PSUM; alpha-mul + reduce on DVE gives scores;
    softmax denominator deferred: e = exp(scores) unnormalized, the final
    result is (sum_l e*af) * 1/(sum_l e).
  - weighted sum over l is a matmul per (tile, d-chunk) using masked lhsT
    columns: e_cols[:, b] = e * indicator(row belongs to b); the denominator
    reuses e_cols against a constant [1, 0] column pair (free dim 2 keeps
    the 16-bit matmul free-dim rule happy).
  - the loop is software-pipelined with a 2-tile skew: iteration t runs the
    DVE/ACT score chain for tile t+1, the PSUM broadcast+inject for tile
    t+2, and the PE accumulation for tile t, so each engine's in-order
    queue always has ready work (the PE p-state ramp needs ~3 us of
    continuous busy to reach full clock).
"""

import os
from contextlib import ExitStack

import numpy as np
import ml_dtypes

import concourse.bass as bass
import concourse.mybir as mybir
import concourse.tile as tile
from concourse import bacc
from concourse.bass_utils import run_bass_kernel_spmd

F32 = mybir.dt.float32
BF16 = mybir.dt.bfloat16
AF_T = mybir.ActivationFunctionType
NPBF16 = ml_dtypes.bfloat16

B, L, D, A = 128, 196, 2048, 512
NCORES = 8
BPC = B // NCORES          # 16 batch rows per core
R = BPC * L                # 3136 (b, l) rows per core
P = 128                    # partitions
NT = (R + P - 1) // P      # 25 row tiles (24 full + one 64-row tail)
NFULL_T = R // P           # 24
TAILR = R - NFULL_T * P    # 64
KCH = D // P               # 16 k-chunks for the h @ W.T matmul
DCH = 4                    # d chunks of 512 for the weighted sum
DC = D // DCH              # 512
AFG = 5                    # row tiles per af/iaf DMA group (25 = 5*5)
NGRP = NT // AFG           # 5


def _ptile(t):
    return P if t < NT - 1 else TAILR


def _build_program():
    nc = bacc.Bacc(None, target_bir_lowering=False)

    h_t = nc.declare_dram_parameter("h_t", [P, KCH * BPC], BF16, isOutput=False)
    w_t = nc.declare_dram_parameter("w_t", [P, KCH * A], BF16, isOutput=False)
    b_bc = nc.declare_dram_parameter("b_bc", [BPC, A], BF16, isOutput=False)
    alpha_bc = nc.declare_dram_parameter("alpha_bc", [P, A], BF16, isOutput=False)
    alphab_bc = nc.declare_dram_parameter("alphab_bc", [P, 1], F32, isOutput=False)
    ident = nc.declare_dram_parameter("ident", [P, P], BF16, isOutput=False)
    ind = nc.declare_dram_parameter("ind", [P, NT * BPC], BF16, isOutput=False)
    ind_t = nc.declare_dram_parameter("ind_t", [BPC, R], BF16, isOutput=False)
    iaf = nc.declare_dram_parameter("iaf", [P, NT * A], BF16, isOutput=False)
    af = nc.declare_dram_parameter("af", [P, NT * D], BF16, isOutput=False)
    out = nc.declare_dram_parameter("out", [BPC, D], F32, isOutput=True)

    with ExitStack() as ctx:
        tc = ctx.enter_context(tile.TileContext(nc))
        consts = ctx.enter_context(tc.tile_pool(name="consts", bufs=1))
        wpool = ctx.enter_context(tc.tile_pool(name="wpool", bufs=1))
        iafp = ctx.enter_context(tc.tile_pool(name="iafp", bufs=1))
        afp = ctx.enter_context(tc.tile_pool(name="afp", bufs=3))
        scr = ctx.enter_context(tc.tile_pool(name="scr", bufs=2))
        ps_bc = ctx.enter_context(
            tc.tile_pool(name="ps_bc", bufs=3, space=bass.MemorySpace.PSUM)
        )
        ps_acc = ctx.enter_context(
            tc.tile_pool(name="ps_acc", bufs=1, space=bass.MemorySpace.PSUM)
        )

        # --- constants / weights (all partition-contiguous in DRAM) ---
        ht_sb = consts.tile([P, KCH * BPC], BF16)
        nc.sync.dma_start(ht_sb[:], h_t[:, :])
        w_sb = wpool.tile([P, KCH * A], BF16)
        nc.sync.dma_start(w_sb[:], w_t[:, :])
        bbc_sb = consts.tile([BPC, A], BF16)
        nc.sync.dma_start(bbc_sb[:], b_bc[:, :])
        abc_sb = consts.tile([P, A], BF16)
        nc.sync.dma_start(abc_sb[:], alpha_bc[:, :])
        abb_sb = consts.tile([P, 1], F32)
        nc.sync.dma_start(abb_sb[:], alphab_bc[:, :])
        ident_sb = consts.tile([P, P], BF16)
        nc.sync.dma_start(ident_sb[:], ident[:, :])
        ind_sb = consts.tile([P, NT * BPC], BF16)
        nc.sync.dma_start(ind_sb[:], ind[:, :])
        indt_sb = consts.tile([BPC, R], BF16)
        nc.sync.dma_start(indt_sb[:], ind_t[:, :])

        ones2_sb = consts.tile([P, 2], BF16)
        nc.gpsimd.memset(ones2_sb[:, 0:1], 1.0)
        nc.gpsimd.memset(ones2_sb[:, 1:2], 0.0)

        scores_all = consts.tile([P, NT], F32)
        e_all = consts.tile([P, NT], F32)

        # --- iaf: fully SBUF-resident (3.2 MB bf16), loaded in 5-tile chunks
        # interleaved with the af groups so early tiles are ready fast ---
        iaf_all = iafp.tile([P, NT * A], BF16)
        af_groups = {}

        def issue_group(g):
            t0 = g * AFG
            nc.sync.dma_start(
                iaf_all[:, t0 * A : (t0 + AFG) * A], iaf[:, t0 * A : (t0 + AFG) * A]
            )
            af_g = afp.tile([P, AFG * D], BF16, tag="af")
            nc.sync.dma_start(af_g[:], af[:, t0 * D : (t0 + AFG) * D])
            af_groups[g] = af_g

        issue_group(0)
        issue_group(1)

        # --- att_hb = h @ W.T + b_h, shape [BPC, A] ---
        atthb_ps = ps_bc.tile([BPC, A], F32, tag="bc")
        for k in range(KCH):
            nc.tensor.matmul(
                atthb_ps[:],
                ht_sb[:, k * BPC : (k + 1) * BPC],
                w_sb[:, k * A : (k + 1) * A],
                start=(k == 0),
                stop=(k == KCH - 1),
            )
        atthb_sb = consts.tile([BPC, A], BF16)
        nc.vector.tensor_add(atthb_sb[:], atthb_ps[:], bbc_sb[:])

        # --- accumulators for the weighted sum and softmax denominator ---
        acc_ps = ps_acc.tile([BPC, DCH, DC], F32)
        sums_ps = ps_acc.tile([BPC, 2], F32)

        bc_tiles = {}

        def bc_inject(t):
            """tile rows of att_hb + iaf, accumulated in one PSUM group."""
            pt = _ptile(t)
            bc_ps = ps_bc.tile([P, A], F32, tag="bc")
            nc.tensor.matmul(
                bc_ps[:pt, :],
                indt_sb[:, t * P : t * P + pt],
                atthb_sb[:],
                start=True,
                stop=False,
            )
            nc.tensor.matmul(
                bc_ps[:pt, :],
                ident_sb[:pt, :pt],
                iaf_all[:pt, t * A : (t + 1) * A],
                start=False,
                stop=True,
            )
            bc_tiles[t] = bc_ps

        ecols_tiles = {}

        def chain(t):
            """tanh -> alpha-mul -> reduce -> exp -> masked e columns."""
            pt = _ptile(t)
            bc_ps = bc_tiles.pop(t)
            tanh = scr.tile([P, A], BF16, tag="tanh")
            nc.scalar.activation(tanh[:pt, :], bc_ps[:pt, :], AF_T.Tanh)
            ttr_out = scr.tile([P, A], BF16, tag="ttr")
            nc.vector.tensor_mul(ttr_out[:pt, :], tanh[:pt, :], abc_sb[:pt, :])
            nc.vector.tensor_reduce(
                scores_all[:pt, t : t + 1],
                ttr_out[:pt, :],
                axis=mybir.AxisListType.X,
                op=mybir.AluOpType.add,
            )
            nc.scalar.activation(
                e_all[:pt, t : t + 1],
                scores_all[:pt, t : t + 1],
                AF_T.Exp,
                bias=abb_sb[:pt, :],
            )
            ecols = scr.tile([P, BPC], BF16, tag="ecols")
            nc.vector.tensor_scalar_mul(
                ecols[:pt, :],
                ind_sb[:pt, t * BPC : (t + 1) * BPC],
                e_all[:pt, t : t + 1],
            )
            ecols_tiles[t] = ecols

        bc_inject(0)
        bc_inject(1)
        chain(0)

        for t in range(NT):
            pt = _ptile(t)
            if t > 0 and t % AFG == 0 and t // AFG + 1 < NGRP:
                issue_group(t // AFG + 1)
            if t + 1 < NT:
                chain(t + 1)
            if t + 2 < NT:
                bc_inject(t + 2)

            af_g = af_groups[t // AFG]
            ecols = ecols_tiles.pop(t)
            for c in range(DCH):
                nc.tensor.matmul(
                    acc_ps[:, c, :],
                    ecols[:pt, :],
                    af_g[:pt, (t % AFG) * D + c * DC : (t % AFG) * D + (c + 1) * DC],
                    start=(t == 0),
                    stop=(t == NT - 1),
                )
            # denominator: sums[b] = sum_rows e_cols[:, b]
            nc.tensor.matmul(
                sums_ps[:],
                ecols[:pt, :],
                ones2_sb[:pt, :],
                start=(t == 0),
                stop=(t == NT - 1),
            )

        # --- normalize and store ---
        recip = consts.tile([BPC, 1], F32)
        nc.vector.reciprocal(recip[:], sums_ps[:, 0:1])
        out_sb = consts.tile([BPC, D], F32)
        nc.scalar.mul(
            out_sb[:, :].rearrange("b (c d) -> b c d", c=DCH), acc_ps[:, :, :], recip[:]
        )
        nc.sync.dma_start(out[:, :], out_sb[:])

    nc.compile()
    return nc


_PROGRAM = None


def _get_program():
    global _PROGRAM
    if _PROGRAM is None:
        _PROGRAM = _build_program()
    return _PROGRAM


def _pack_rows(x16, ncols):
    """[R, C] bf16 -> [P, NT*C] with dev[p, t*C:(t+1)*C] = x[t*P+p], zero pad."""
    dev = np.zeros((P, NT, ncols), NPBF16)
    dev[:, :NFULL_T, :] = x16[: NFULL_T * P].reshape(NFULL_T, P, ncols).transpose(1, 0, 2)
    dev[:TAILR, NFULL_T, :] = x16[NFULL_T * P :]
    return np.ascontiguousarray(dev.reshape(P, NT * ncols))


def _host_prep(h, att_feats, internal_att_feats, h2att_w, h2att_b, alpha_w, alpha_b):
    h16 = np.asarray(h).astype(NPBF16)
    af16 = np.asarray(att_feats).astype(NPBF16).reshape(B, L * D)
    iaf16 = np.asarray(internal_att_feats).astype(NPBF16).reshape(B, L * A)
    w16 = np.asarray(h2att_w).astype(NPBF16)                    # [A, D]
    h2att_b = np.asarray(h2att_b, np.float32)
    alpha_w = np.asarray(alpha_w, np.float32)
    alpha_b = np.asarray(alpha_b, np.float32)

    # w_dev[p, k*A+a] = W[a, k*P+p]
    w_dev = np.ascontiguousarray(
        w16.T.reshape(KCH, P, A).transpose(1, 0, 2).reshape(P, KCH * A)
    )
    b_bc = np.tile(h2att_b.reshape(1, A).astype(NPBF16), (BPC, 1))
    alpha_bc = np.tile(alpha_w.reshape(1, A).astype(NPBF16), (P, 1))
    alphab_bc = np.full((P, 1), float(alpha_b.reshape(-1)[0]), np.float32)
    ident = np.eye(P, dtype=NPBF16)

    ind_rows = np.zeros((NT * P, BPC), np.float32)
    rows = np.arange(R)
    ind_rows[rows, rows // L] = 1.0
    ind_dev = np.ascontiguousarray(
        ind_rows.reshape(NT, P, BPC).transpose(1, 0, 2).reshape(P, NT * BPC)
    ).astype(NPBF16)
    ind_t = np.ascontiguousarray(ind_rows[:R].T).astype(NPBF16)  # [BPC, R]

    in_maps = []
    for i in range(NCORES):
        sl = slice(i * BPC, (i + 1) * BPC)
        # h_t_dev[p, k*BPC+b] = h[b, k*P+p]
        ht_dev = np.ascontiguousarray(
            h16[sl].T.reshape(KCH, P, BPC).transpose(1, 0, 2).reshape(P, KCH * BPC)
        )
        in_maps.append(
            {
                "h_t": ht_dev,
                "w_t": w_dev,
                "b_bc": b_bc,
                "alpha_bc": alpha_bc,
                "alphab_bc": alphab_bc,
                "ident": ident,
                "ind": ind_dev,
                "ind_t": ind_t,
                "iaf": _pack_rows(iaf16[sl].reshape(R, A), A),
                "af": _pack_rows(af16[sl].reshape(R, D), D),
            }
        )
    return in_maps


def run(trace=False, **inputs):
    """Run the SPMD kernel; returns (full_output [B, D], BassKernelResults)."""
    nc = _get_program()
    in_maps = _host_prep(**inputs)
    res = run_bass_kernel_spmd(nc, in_maps, list(range(NCORES)), trace=trace)
    out = np.concatenate([res.results[i]["out"] for i in range(NCORES)], axis=0)
    return out, res


def kernel(**inputs):
    out, _ = run(trace=False, **inputs)
    return out
